# revision 16
# baseline (speedup 1.0000x reference)
# Trainium2 Bass kernel for: ConvTranspose2d(64->128, k=4, stride=1) -> spatial
# mean -> +biases -> 10*logsumexp over channels.
#
# Math: with full (K-1) output padding, the mean over the ENTIRE conv-transpose
# output spatial extent sees every input pixel through all K*K taps, so
#   pooled[n,co] = (sum_hw x[n,ci,hw]) @ (sum_kk w[ci,co,kk]) / (Ho*Wo) + cb + eb
# exactly. The conv collapses to a spatial sum + a (Cin x Cout) matmul.
#
# Sharding: data-parallel over batch N=32 across 8 cores (4 batches/core),
# params replicated.
#
# v7 (v1 31.65us, v2 28.88, v3 31.34, v4 24.99, v5 25.64, v6 30.80):
# - x ships bf16 (rel err ~1e-4 vs the 2e-2 gate) as 8 HWDGE chunks on the
#   SP ring, issued back-to-back with NO reduce-gating: queued transfers
#   share SDMA bandwidth round-robin and complete in issue order; gating
#   issues on reduces (v2/v3) collapses BW to ~170; SWDGE accumulate-DMAs
#   (v6) serialize at ~93 GB/s per transfer.
# - Chunk sizes per block [1536, 1536, 512, 512], blocks interleaved, big
#   chunks first: early chunks maximize queue depth (BW ramps with depth),
#   small last chunks keep the final reduce trail short.
# - Spatial-sum reduces split DVE (tensor_scalar+accum_out, ~1.08ns/col) /
#   ACT (Copy+accum_out, ~(cols+352)/1.2 + 278ns accumulator read); both
#   accumulate fp32. Assignment balances each engine's chain ~5.5us.
# - ENTIRE weight pipeline on the otherwise-idle PE: host packs w as 8
#   slabs wsl_j[t*64+ci, co] = w[ci, co, 2j+t]; 8 PSUM-accumulated matmuls
#   against the host 0/1 dup matrix D (D[p,m] = p%64 == m%64) give
#   wdup[m, co] = sum_k w[m%64, co, k] directly; one ACT copy PSUM->SBUF.
#   DVE does no weight work at all.
# - One fused param DMA (w-slabs | D | biases, bf16) on the ACT HWDGE ring:
#   10 DMAs total, so Tile's 8 DMAHW sem lanes only recycle for the last
#   x chunk and y, both long after their lane's previous completion.
# - Manual LoadActFuncSet("natural_log_exp_and_others": Copy+Exp+Ln) as the
#   FIRST Scalar instruction (any earlier Scalar op makes the
#   insert_act_table_loads pass emit a spurious second 1.28us table load).
# - Tail: pooled computed TRANSPOSED [co, n] so the 1/(Ho*Wo) scale and both
#   biases fold into the Exp activation (func(scale*in+bias), bias = fp32
#   per-partition AP over co); channel sum = tiny PE matmul against ones;
#   Ln reads PSUM; x10 on DVE; 16-byte result DMA on the SP ring.

import os

import numpy as np
import ml_dtypes

import concourse.bacc as bacc
import concourse.bass as bass
import concourse.mybir as mybir
import concourse.tile as tile
from concourse.bass_utils import run_bass_kernel_spmd
from concourse.hw_specs import get_activation_tables

N, CIN, COUT, K, H, W = 32, 64, 128, 4, 64, 64
NCORES = 8
NLOC = N // NCORES          # 4 batches per core
HW = H * W                  # 4096
ROWS = NLOC * CIN           # 256 rows (n,ci) per core
RBLK = ROWS // 128          # 2 row blocks of 128 partitions
CHUNKS = [512, 1792, 1792]  # per-block chunk widths: SMALL chunk first so
                            # the first completion (and hence the reduce
                            # pipeline) starts ~1.3us earlier; aggregate DMA
                            # BW is set by total queued bytes, not shape
NCHUNK = len(CHUNKS)
NCK = RBLK * NCHUNK         # 4 chunks
VFRAC = 0.5625              # DVE's column share of each chunk (ACT gets rest);
                            # DVE consumes its share as tensor_tensor_reduce
                            # (two half-slices added elementwise, then
                            # accumulated) = 2 columns/cycle vs 1 for plain
                            # tensor_scalar+accum
SCALE = 1.0 / float((H + K - 1) * (W + K - 1))   # 1/4489
NSLAB = K * K // 2          # 8 weight slabs of [128, COUT]
WCOLS = COUT * NSLAB        # 1024 weight cols
PCOLS = WCOLS + 128 + 2     # fused param width: w-slabs | dup | biases

F32 = mybir.dt.float32
BF16 = mybir.dt.bfloat16
FP8 = mybir.dt.float8e4
BF16_NP = ml_dtypes.bfloat16
FP8_NP = ml_dtypes.float8_e4m3fn


_CACHE: dict = {}


def _build_module() -> bacc.Bacc:
    nc = bacc.Bacc("TRN2", target_bir_lowering=False, enable_partition_id=False)

    x_d = nc.dram_tensor("xc", [ROWS, HW], FP8, kind="ExternalInput").ap()
    w_d = nc.dram_tensor("w", [128, PCOLS], BF16, kind="ExternalInput").ap()
    y_d = nc.dram_tensor("y", [1, NLOC], F32, kind="ExternalOutput").ap()

    with tile.TileContext(nc) as tc:
        with (
            tc.tile_pool(name="xpool", bufs=1) as xpool,
            tc.tile_pool(name="small", bufs=1) as small,
            tc.tile_pool(name="psw", bufs=1, space="PSUM") as psw_pool,
            tc.tile_pool(name="ps2", bufs=1, space="PSUM") as ps2_pool,
            tc.tile_pool(name="ps3", bufs=1, space="PSUM") as ps3_pool,
        ):
            # ---- ACT table preload MUST be the first Scalar instruction ----
            act_tables = get_activation_tables(nc.m.arch)
            set_id = next(
                i
                for i, (_, funcs) in enumerate(act_tables.items())
                if mybir.ActivationFunctionType.Exp in funcs
                and mybir.ActivationFunctionType.Ln in funcs
                and mybir.ActivationFunctionType.Copy in funcs
            )
            nc.scalar.add_instruction(
                mybir.InstLoadActFuncSet(
                    name=nc.get_next_instruction_name(), act_func_set_id=set_id
                )
            )

            # ---- fused param DMA on the ACT HWDGE ring ----
            wd_t = small.tile([128, PCOLS], BF16)
            nc.scalar.dma_start(out=wd_t, in_=w_d)
            dmat = wd_t[:, WCOLS : WCOLS + 128]
            bsrows = wd_t[:, WCOLS + 128 : WCOLS + 130]

            # ---- small constants (DVE, early) ----
            s2m = small.tile([128, NLOC], BF16)
            nc.vector.memset(s2m, 0.0)
            onesb = small.tile([128, 1], BF16)
            nc.vector.memset(onesb, 1.0)
            biasc = small.tile([COUT, 1], F32)
            nc.vector.reduce_sum(out=biasc, in_=bsrows, axis=mybir.AxisListType.X)

            # ---- wdup[m,co] = sum_k w[m%64, co, k] via 8 PSUM-accumulated
            # PE matmuls of the w-slabs against the dup matrix D ----
            psw = psw_pool.tile([128, COUT], F32, space="PSUM")
            for j in range(NSLAB):
                nc.tensor.matmul(
                    out=psw,
                    lhsT=dmat,
                    rhs=wd_t[:, j * COUT : (j + 1) * COUT],
                    start=(j == 0),
                    stop=(j == NSLAB - 1),
                )
            wdup = small.tile([128, COUT], BF16)
            nc.scalar.activation(
                out=wdup, in_=psw, func=mybir.ActivationFunctionType.Copy
            )

            # ---- x stream: every completed chunk is reduced by BOTH
            # engines at once in column slices (DVE 1152 / ACT 896) ----
            parts = small.tile([128, 2 * NCK], F32)
            scrV = small.tile([128, max(CHUNKS)], FP8)
            scrA = small.tile([128, max(CHUNKS)], BF16)
            col0 = [0]
            for w_ in CHUNKS[:-1]:
                col0.append(col0[-1] + w_)

            for idx in range(NCK):
                r, c = idx % 2, idx // 2
                cw = CHUNKS[c]
                xt = xpool.tile([128, cw], FP8, tag=f"xt{idx}")
                nc.sync.dma_start(
                    out=xt,
                    in_=x_d[r * 128 : (r + 1) * 128, col0[c] : col0[c] + cw],
                )
                base = 2 * (r * NCHUNK + c)
                vslc = (int(cw * VFRAC) + 31) & ~31
                nc.vector.tensor_scalar(
                    out=scrV[:, 0:vslc],
                    in0=xt[:, 0:vslc],
                    scalar1=0.0,
                    scalar2=None,
                    op0=mybir.AluOpType.add,
                    op1=mybir.AluOpType.add,
                    accum_out=parts[:, base : base + 1],
                )
                nc.scalar.activation(
                    out=scrA[:, 0 : cw - vslc],
                    in_=xt[:, vslc:cw],
                    func=mybir.ActivationFunctionType.Copy,
                    accum_out=parts[:, base + 1 : base + 2],
                )

            # ---- combine partials: s2[p, r] ----
            s2 = small.tile([128, RBLK], F32)
            nc.vector.reduce_sum(
                out=s2,
                in_=parts.rearrange("p (r c) -> p r c", r=RBLK),
                axis=mybir.AxisListType.X,
            )

            # ---- masked rhs (128, 4) bf16: s2m[(n%2)*64 + ci, n] = S[n, ci]
            s2m_v = s2m.rearrange("p (r t) -> p r t", t=2)
            s2_v = s2.rearrange("p (r t) -> p r t", t=1)
            nc.vector.tensor_copy(s2m_v[0:64, :, 0:1], s2_v[0:64, :, :])
            nc.vector.tensor_copy(s2m_v[64:128, :, 1:2], s2_v[64:128, :, :])

            # ---- pooled^T (co, n) in PSUM via one bf16 matmul ----
            pooledT = ps2_pool.tile([COUT, NLOC], F32, space="PSUM")
            nc.tensor.matmul(out=pooledT, lhsT=wdup, rhs=s2m, start=True, stop=True)

            # ---- exp(SCALE * pooledT + (cb+eb)[co]) -> bf16 SBUF ----
            expT = small.tile([COUT, NLOC], BF16)
            nc.scalar.activation(
                out=expT,
                in_=pooledT,
                func=mybir.ActivationFunctionType.Exp,
                bias=biasc,
                scale=float(SCALE),
            )

            # ---- sum over channels (partition axis) via PE against ones ----
            sume = ps3_pool.tile([1, NLOC], F32, space="PSUM")
            nc.tensor.matmul(out=sume, lhsT=onesb, rhs=expT, start=True, stop=True)

            # ---- 10 * ln(sum) ----
            logv = small.tile([1, NLOC], F32)
            nc.scalar.activation(
                out=logv, in_=sume, func=mybir.ActivationFunctionType.Ln
            )
            outv = small.tile([1, NLOC], F32)
            nc.vector.tensor_scalar_mul(out=outv, in0=logv, scalar1=10.0)
            nc.sync.dma_start(out=y_d, in_=outv)

    nc.compile()
    return nc


def kernel(x, weight, conv_bias, extra_bias):
    x = np.asarray(x, dtype=np.float32)
    weight = np.asarray(weight, dtype=np.float32)
    conv_bias = np.asarray(conv_bias, dtype=np.float32)
    extra_bias = np.asarray(extra_bias, dtype=np.float32)
    assert x.shape == (N, CIN, H, W), x.shape
    assert weight.shape == (CIN, COUT, K, K), weight.shape

    if "nc" not in _CACHE:
        _CACHE["nc"] = _build_module()
    nc = _CACHE["nc"]

    # w slabs: wsl[j][t*64+ci, co] = w[ci, co, k=2j+t], flattened [128, 1024]
    # with col = j*COUT + co
    wsl = (
        weight.reshape(CIN, COUT, NSLAB, 2)   # [ci, co, j, t]
        .transpose(3, 0, 2, 1)                # [t, ci, j, co]
        .reshape(128, NSLAB, COUT)
        .reshape(128, WCOLS)
        .astype(BF16_NP)
    )
    dmat = (
        np.arange(128)[:, None] % 64 == np.arange(128)[None, :] % 64
    ).astype(BF16_NP)
    bs2 = np.zeros((128, 2), dtype=BF16_NP)
    bs2[:COUT, 0] = conv_bias.astype(BF16_NP)
    bs2[:COUT, 1] = extra_bias.astype(BF16_NP)
    wd = np.ascontiguousarray(np.concatenate([wsl, dmat, bs2], axis=1))
    xb = x.astype(FP8_NP)
    in_maps = []
    for c in range(NCORES):
        xc = np.ascontiguousarray(xb[c * NLOC : (c + 1) * NLOC].reshape(ROWS, HW))
        in_maps.append({"xc": xc, "w": wd})

    trace = os.environ.get("BASS_KERNEL_TRACE") == "1"
    res = run_bass_kernel_spmd(
        nc, in_maps, core_ids=list(range(NCORES)), trace=trace
    )
    _CACHE["last_result"] = res
    # each core returns y[1, NLOC]; stack -> (NCORES, NLOC) -> (N, 1)
    return np.concatenate([r["y"] for r in res.results], axis=0).reshape(N, 1)


# revision 17
# speedup vs baseline: 1.0433x; 1.0433x over previous
# Trainium2 Bass kernel for: ConvTranspose2d(64->128, k=4, stride=1) -> spatial
# mean -> +biases -> 10*logsumexp over channels.
#
# Math: with full (K-1) output padding, the mean over the ENTIRE conv-transpose
# output spatial extent sees every input pixel through all K*K taps, so
#   pooled[n,co] = (sum_hw x[n,ci,hw]) @ (sum_kk w[ci,co,kk]) / (Ho*Wo) + cb + eb
# exactly. The conv collapses to a spatial sum + a (Cin x Cout) matmul.
#
# Sharding: data-parallel over batch N=32 across 8 cores (4 batches/core),
# params replicated.
#
# v7 (v1 31.65us, v2 28.88, v3 31.34, v4 24.99, v5 25.64, v6 30.80):
# - x ships bf16 (rel err ~1e-4 vs the 2e-2 gate) as 8 HWDGE chunks on the
#   SP ring, issued back-to-back with NO reduce-gating: queued transfers
#   share SDMA bandwidth round-robin and complete in issue order; gating
#   issues on reduces (v2/v3) collapses BW to ~170; SWDGE accumulate-DMAs
#   (v6) serialize at ~93 GB/s per transfer.
# - Chunk sizes per block [1536, 1536, 512, 512], blocks interleaved, big
#   chunks first: early chunks maximize queue depth (BW ramps with depth),
#   small last chunks keep the final reduce trail short.
# - Spatial-sum reduces split DVE (tensor_scalar+accum_out, ~1.08ns/col) /
#   ACT (Copy+accum_out, ~(cols+352)/1.2 + 278ns accumulator read); both
#   accumulate fp32. Assignment balances each engine's chain ~5.5us.
# - ENTIRE weight pipeline on the otherwise-idle PE: host packs w as 8
#   slabs wsl_j[t*64+ci, co] = w[ci, co, 2j+t]; 8 PSUM-accumulated matmuls
#   against the host 0/1 dup matrix D (D[p,m] = p%64 == m%64) give
#   wdup[m, co] = sum_k w[m%64, co, k] directly; one ACT copy PSUM->SBUF.
#   DVE does no weight work at all.
# - One fused param DMA (w-slabs | D | biases, bf16) on the ACT HWDGE ring:
#   10 DMAs total, so Tile's 8 DMAHW sem lanes only recycle for the last
#   x chunk and y, both long after their lane's previous completion.
# - Manual LoadActFuncSet("natural_log_exp_and_others": Copy+Exp+Ln) as the
#   FIRST Scalar instruction (any earlier Scalar op makes the
#   insert_act_table_loads pass emit a spurious second 1.28us table load).
# - Tail: pooled computed TRANSPOSED [co, n] so the 1/(Ho*Wo) scale and both
#   biases fold into the Exp activation (func(scale*in+bias), bias = fp32
#   per-partition AP over co); channel sum = tiny PE matmul against ones;
#   Ln reads PSUM; x10 on DVE; 16-byte result DMA on the SP ring.

import os

import numpy as np
import ml_dtypes

import concourse.bacc as bacc
import concourse.bass as bass
import concourse.mybir as mybir
import concourse.tile as tile
from concourse.bass_utils import run_bass_kernel_spmd
from concourse.hw_specs import get_activation_tables

N, CIN, COUT, K, H, W = 32, 64, 128, 4, 64, 64
NCORES = 8
NLOC = N // NCORES          # 4 batches per core
HW = H * W                  # 4096
ROWS = NLOC * CIN           # 256 rows (n,ci) per core
RBLK = ROWS // 128          # 2 row blocks of 128 partitions
CHUNKS = [2048, 2048]       # per-block chunk widths: 4KB/partition DMA lines
                            # (aggregate BW scales with line size: 2-3KB
                            # lines measured ~240 GB/s, 4KB ~300+)
NCHUNK = len(CHUNKS)
NCK = RBLK * NCHUNK         # 4 chunks
VSLC = 1088                 # DVE's column share of each chunk (ACT gets rest)
SCALE = 1.0 / float((H + K - 1) * (W + K - 1))   # 1/4489
NSLAB = K * K // 2          # 8 weight slabs of [128, COUT]
WCOLS = COUT * NSLAB        # 1024 weight cols
PCOLS = WCOLS + 128 + 2     # fused param width: w-slabs | dup | biases

F32 = mybir.dt.float32
BF16 = mybir.dt.bfloat16
FP8 = mybir.dt.float8e4
BF16_NP = ml_dtypes.bfloat16
FP8_NP = ml_dtypes.float8_e4m3fn


_CACHE: dict = {}


def _build_module() -> bacc.Bacc:
    nc = bacc.Bacc("TRN2", target_bir_lowering=False, enable_partition_id=False)

    x_d = nc.dram_tensor("xc", [ROWS, HW], FP8, kind="ExternalInput").ap()
    w_d = nc.dram_tensor("w", [128, PCOLS], BF16, kind="ExternalInput").ap()
    y_d = nc.dram_tensor("y", [1, NLOC], F32, kind="ExternalOutput").ap()

    with tile.TileContext(nc) as tc:
        with (
            tc.tile_pool(name="xpool", bufs=1) as xpool,
            tc.tile_pool(name="small", bufs=1) as small,
            tc.tile_pool(name="psw", bufs=1, space="PSUM") as psw_pool,
            tc.tile_pool(name="ps2", bufs=1, space="PSUM") as ps2_pool,
            tc.tile_pool(name="ps3", bufs=1, space="PSUM") as ps3_pool,
        ):
            # ---- ACT table preload MUST be the first Scalar instruction ----
            act_tables = get_activation_tables(nc.m.arch)
            set_id = next(
                i
                for i, (_, funcs) in enumerate(act_tables.items())
                if mybir.ActivationFunctionType.Exp in funcs
                and mybir.ActivationFunctionType.Ln in funcs
                and mybir.ActivationFunctionType.Copy in funcs
            )
            nc.scalar.add_instruction(
                mybir.InstLoadActFuncSet(
                    name=nc.get_next_instruction_name(), act_func_set_id=set_id
                )
            )

            # ---- fused param DMA on the ACT HWDGE ring ----
            wd_t = small.tile([128, PCOLS], BF16)
            nc.scalar.dma_start(out=wd_t, in_=w_d)
            dmat = wd_t[:, WCOLS : WCOLS + 128]
            bsrows = wd_t[:, WCOLS + 128 : WCOLS + 130]

            # ---- small constants (DVE, early) ----
            s2m = small.tile([128, NLOC], BF16)
            nc.vector.memset(s2m, 0.0)
            onesb = small.tile([128, 1], BF16)
            nc.vector.memset(onesb, 1.0)
            biasc = small.tile([COUT, 1], F32)
            nc.vector.reduce_sum(out=biasc, in_=bsrows, axis=mybir.AxisListType.X)

            # ---- wdup[m,co] = sum_k w[m%64, co, k] via 8 PSUM-accumulated
            # PE matmuls of the w-slabs against the dup matrix D ----
            psw = psw_pool.tile([128, COUT], F32, space="PSUM")
            for j in range(NSLAB):
                nc.tensor.matmul(
                    out=psw,
                    lhsT=dmat,
                    rhs=wd_t[:, j * COUT : (j + 1) * COUT],
                    start=(j == 0),
                    stop=(j == NSLAB - 1),
                )
            wdup = small.tile([128, COUT], BF16)
            nc.scalar.activation(
                out=wdup, in_=psw, func=mybir.ActivationFunctionType.Copy
            )

            # ---- x stream: every completed chunk is reduced by BOTH
            # engines at once in column slices (DVE 1152 / ACT 896) ----
            parts = small.tile([128, 2 * NCK], F32)
            scrV = small.tile([128, VSLC], BF16)
            scrA = small.tile([128, max(CHUNKS) - VSLC], BF16)
            col0 = [0]
            for w_ in CHUNKS[:-1]:
                col0.append(col0[-1] + w_)

            for idx in range(NCK):
                r, c = idx % 2, idx // 2
                cw = CHUNKS[c]
                xt = xpool.tile([128, cw], FP8, tag=f"xt{idx}")
                nc.sync.dma_start(
                    out=xt,
                    in_=x_d[r * 128 : (r + 1) * 128, col0[c] : col0[c] + cw],
                )
                base = 2 * (r * NCHUNK + c)
                nc.vector.tensor_scalar(
                    out=scrV,
                    in0=xt[:, 0:VSLC],
                    scalar1=0.0,
                    scalar2=None,
                    op0=mybir.AluOpType.add,
                    op1=mybir.AluOpType.add,
                    accum_out=parts[:, base : base + 1],
                )
                nc.scalar.activation(
                    out=scrA[:, 0 : cw - VSLC],
                    in_=xt[:, VSLC:cw],
                    func=mybir.ActivationFunctionType.Copy,
                    accum_out=parts[:, base + 1 : base + 2],
                )

            # ---- combine partials: s2[p, r] ----
            s2 = small.tile([128, RBLK], F32)
            nc.vector.reduce_sum(
                out=s2,
                in_=parts.rearrange("p (r c) -> p r c", r=RBLK),
                axis=mybir.AxisListType.X,
            )

            # ---- masked rhs (128, 4) bf16: s2m[(n%2)*64 + ci, n] = S[n, ci]
            s2m_v = s2m.rearrange("p (r t) -> p r t", t=2)
            s2_v = s2.rearrange("p (r t) -> p r t", t=1)
            nc.vector.tensor_copy(s2m_v[0:64, :, 0:1], s2_v[0:64, :, :])
            nc.vector.tensor_copy(s2m_v[64:128, :, 1:2], s2_v[64:128, :, :])

            # ---- pooled^T (co, n) in PSUM via one bf16 matmul ----
            pooledT = ps2_pool.tile([COUT, NLOC], F32, space="PSUM")
            nc.tensor.matmul(out=pooledT, lhsT=wdup, rhs=s2m, start=True, stop=True)

            # ---- exp(SCALE * pooledT + (cb+eb)[co]) -> bf16 SBUF ----
            expT = small.tile([COUT, NLOC], BF16)
            nc.scalar.activation(
                out=expT,
                in_=pooledT,
                func=mybir.ActivationFunctionType.Exp,
                bias=biasc,
                scale=float(SCALE),
            )

            # ---- sum over channels (partition axis) via PE against ones ----
            sume = ps3_pool.tile([1, NLOC], F32, space="PSUM")
            nc.tensor.matmul(out=sume, lhsT=onesb, rhs=expT, start=True, stop=True)

            # ---- 10 * ln(sum) ----
            logv = small.tile([1, NLOC], F32)
            nc.scalar.activation(
                out=logv, in_=sume, func=mybir.ActivationFunctionType.Ln
            )
            outv = small.tile([1, NLOC], F32)
            nc.vector.tensor_scalar_mul(out=outv, in0=logv, scalar1=10.0)
            nc.sync.dma_start(out=y_d, in_=outv)

    nc.compile()
    return nc


def kernel(x, weight, conv_bias, extra_bias):
    x = np.asarray(x, dtype=np.float32)
    weight = np.asarray(weight, dtype=np.float32)
    conv_bias = np.asarray(conv_bias, dtype=np.float32)
    extra_bias = np.asarray(extra_bias, dtype=np.float32)
    assert x.shape == (N, CIN, H, W), x.shape
    assert weight.shape == (CIN, COUT, K, K), weight.shape

    if "nc" not in _CACHE:
        _CACHE["nc"] = _build_module()
    nc = _CACHE["nc"]

    # w slabs: wsl[j][t*64+ci, co] = w[ci, co, k=2j+t], flattened [128, 1024]
    # with col = j*COUT + co
    wsl = (
        weight.reshape(CIN, COUT, NSLAB, 2)   # [ci, co, j, t]
        .transpose(3, 0, 2, 1)                # [t, ci, j, co]
        .reshape(128, NSLAB, COUT)
        .reshape(128, WCOLS)
        .astype(BF16_NP)
    )
    dmat = (
        np.arange(128)[:, None] % 64 == np.arange(128)[None, :] % 64
    ).astype(BF16_NP)
    bs2 = np.zeros((128, 2), dtype=BF16_NP)
    bs2[:COUT, 0] = conv_bias.astype(BF16_NP)
    bs2[:COUT, 1] = extra_bias.astype(BF16_NP)
    wd = np.ascontiguousarray(np.concatenate([wsl, dmat, bs2], axis=1))
    xb = x.astype(FP8_NP)
    in_maps = []
    for c in range(NCORES):
        xc = np.ascontiguousarray(xb[c * NLOC : (c + 1) * NLOC].reshape(ROWS, HW))
        in_maps.append({"xc": xc, "w": wd})

    trace = os.environ.get("BASS_KERNEL_TRACE") == "1"
    res = run_bass_kernel_spmd(
        nc, in_maps, core_ids=list(range(NCORES)), trace=trace
    )
    _CACHE["last_result"] = res
    # each core returns y[1, NLOC]; stack -> (NCORES, NLOC) -> (N, 1)
    return np.concatenate([r["y"] for r in res.results], axis=0).reshape(N, 1)
